# revision 4
# baseline (speedup 1.0000x reference)
# ISTFT kernel for Trainium2 (8 NeuronCores, data-parallel over batch).
#
# Math: out[b, s] for s = 256*c + r (chunk c, offset r) is
#   out[c, r] = sum_{j=0..3} sum_C spec[C, c-j] * invbasis[C, 256*j + r]
# i.e. the overlap-add is folded into 4 shifted matmuls accumulating in PSUM.
# invbasis rows 513 (imag DC) and 1025 (imag Nyquist) are exactly zero
# (pinv of a basis with zero rows), so the contraction packs to exactly
# 1024 = 8 chunks of 128 partitions:
#   packed rows 0..512   = real part rows 0..512   (mag*cos(angle))
#   packed rows 513..1023 = imag part freqs 1..511 (mag*sin(angle))
# Output keeps chunks 4..2047 (the reference trims NFFT=1024 samples per side).
import numpy as np

import concourse.bacc as bacc
import concourse.mybir as mybir
import concourse.tile as tile
from concourse.bass_utils import run_bass_kernel_spmd

F32 = mybir.dt.float32
BF16 = mybir.dt.bfloat16
ALU = mybir.AluOpType
ACTF = mybir.ActivationFunctionType

TWO_PI = 6.283185307179586
INV_2PI = 1.0 / TWO_PI
MAGIC = 12582912.0  # 1.5 * 2**23, forces round-to-nearest-int in fp32
PI = 3.141592653589793
HALF_PI = PI / 2
SIN_SCALE = 0.999999  # keeps rounding fuzz at +/-pi inside the Sin table domain

B_LOCAL = 2      # batches per core (16 total / 8 cores)
T = 2048         # STFT frames
NCH = 2048       # output chunks computed (chunk = 256 samples); keep 4..2047
W = 512          # elementwise column-slice width
N_CORES = 8


def _elementwise_pipeline(nc, pool_get, ang_ap, mag_ap, sin_out, cos_out):
    """Range-reduce angle, compute sin/cos on ACT, multiply by mag on DVE.

    ang_ap/mag_ap: f32 SBUF APs of identical shape.
    sin_out: (ap, in_slice) or None  -> mag*sin(angle)
    cos_out: (ap, in_slice) or None  -> mag*cos(angle)
    pool_get(tag) -> fresh f32 tile AP shaped like ang_ap.
    """
    tb = pool_get("tb")
    nc.vector.tensor_scalar(
        out=tb, in0=ang_ap, scalar1=INV_2PI, scalar2=MAGIC, op0=ALU.mult, op1=ALU.add
    )
    kk = pool_get("kk")
    nc.vector.tensor_scalar_sub(kk, tb, MAGIC)
    red = pool_get("red")
    # red = angle - k*2pi  in [-pi, pi] (+tiny rounding fuzz)
    nc.vector.scalar_tensor_tensor(
        out=red, in0=kk, scalar=-TWO_PI, in1=ang_ap, op0=ALU.mult, op1=ALU.add
    )
    if sin_out is not None:
        sv = pool_get("sv")
        nc.scalar.activation(sv, red, ACTF.Sin, scale=SIN_SCALE)
        ap, sl = sin_out
        nc.vector.tensor_mul(ap, mag_ap[sl] if sl else mag_ap, sv[sl] if sl else sv)
    if cos_out is not None:
        redc = pool_get("redc")
        nc.vector.add_range_wrap(
            out=redc, in_=red, shift=HALF_PI, bound=PI, period=TWO_PI
        )
        cv = pool_get("cv")
        nc.scalar.activation(cv, redc, ACTF.Sin, scale=SIN_SCALE)
        ap, sl = cos_out
        nc.vector.tensor_mul(ap, mag_ap[sl] if sl else mag_ap, cv[sl] if sl else cv)


def build_nc():
    nc = bacc.Bacc(target_bir_lowering=False)
    mag = nc.declare_dram_parameter("mag", [B_LOCAL, 513, T], F32, isOutput=False)
    ang = nc.declare_dram_parameter("angle", [B_LOCAL, 513, T], F32, isOutput=False)
    invb = nc.declare_dram_parameter("invbasis", [1026, 1024], F32, isOutput=False)
    out = nc.declare_dram_parameter("out", [B_LOCAL, 523008], F32, isOutput=True)

    with tile.TileContext(nc) as tc:
        with (
            tc.tile_pool(name="const", bufs=1) as constp,
            tc.tile_pool(name="stage", bufs=3) as stagep,
            tc.tile_pool(name="spec", bufs=2) as specp,
            tc.tile_pool(name="work", bufs=3) as workp,
            tc.tile_pool(name="osb", bufs=6) as osbp,
            tc.tile_pool(name="psum", bufs=8, space="PSUM") as psump,
        ):
            # --- invbasis: load f32 chunks, cast to resident bf16 tiles ---
            ib = []
            for q in range(8):
                ibf = stagep.tile([128, 1024], F32, tag="ibf")
                if q < 4:
                    nc.sync.dma_start(out=ibf[:, :], in_=invb[128 * q : 128 * (q + 1), :])
                elif q == 4:
                    nc.sync.dma_start(out=ibf[0:1, :], in_=invb[512:513, :])
                    nc.sync.dma_start(out=ibf[1:128, :], in_=invb[514:641, :])
                else:
                    base = 641 + 128 * (q - 5)
                    nc.sync.dma_start(out=ibf[:, :], in_=invb[base : base + 128, :])
                ibq = constp.tile([128, 1024], BF16, tag=f"ib{q}", name=f"ib{q}")
                nc.vector.tensor_copy(ibq[:, :], ibf[:, :])
                ib.append(ibq)

            for b in range(B_LOCAL):
                # --- spec tiles: bf16 [128, 4 + T], 4 leading zero columns ---
                spec = []
                for q in range(8):
                    st = specp.tile([128, 4 + T], BF16, tag=f"spec{q}", name=f"spec{q}")
                    nc.vector.memset(st[:, 0:4], 0.0)
                    spec.append(st)

                # --- elementwise: rows 0..511 in 4 chunks of 128 ---
                for q in range(4):
                    for s in range(0, T, W):
                        mt = stagep.tile([128, W], F32, tag="mt", name="mt")
                        at = stagep.tile([128, W], F32, tag="at", name="at")
                        rows = slice(128 * q, 128 * (q + 1))
                        nc.sync.dma_start(out=mt[:, :], in_=mag[b, rows, s : s + W])
                        nc.sync.dma_start(out=at[:, :], in_=ang[b, rows, s : s + W])

                        def pool_get(tag):
                            return workp.tile([128, W], F32, tag=tag, name=tag)

                        cs = slice(4 + s, 4 + s + W)
                        # imag rows: freq f -> spec[4+q] partition f%128. For q=0,
                        # partition 0 (freq 0) is garbage here; the row-512 pass
                        # below overwrites it with the real Nyquist-free row 512.
                        _elementwise_pipeline(
                            nc,
                            pool_get,
                            at[:, :],
                            mt,
                            sin_out=(spec[4 + q][:, cs], None),
                            cos_out=(spec[q][:, cs], None),
                        )

                # --- row 512 (real only) ---
                for s in range(0, T, W):
                    m5 = stagep.tile([1, W], F32, tag="m5", name="m5", bufs=2)
                    a5 = stagep.tile([1, W], F32, tag="a5", name="a5", bufs=2)
                    nc.sync.dma_start(out=m5[:, :], in_=mag[b, 512:513, s : s + W])
                    nc.sync.dma_start(out=a5[:, :], in_=ang[b, 512:513, s : s + W])

                    def pool_get5(tag):
                        return workp.tile([1, W], F32, tag=tag + "5", name=tag + "5", bufs=2)

                    _elementwise_pipeline(
                        nc,
                        pool_get5,
                        a5[:, :],
                        m5,
                        sin_out=None,
                        cos_out=(spec[4][0:1, 4 + s : 4 + s + W], None),
                    )

                # --- matmuls with folded overlap-add ---
                for ct in range(16):
                    ps = psump.tile([128, 256], F32, tag="ps", name="ps")
                    c0 = 128 * ct
                    mmi = 0
                    for q in range(8):
                        for j in range(4):
                            nc.tensor.matmul(
                                out=ps[:, :],
                                lhsT=spec[q][:, c0 - j + 4 : c0 - j + 132],
                                rhs=ib[q][:, 256 * j : 256 * (j + 1)],
                                start=(mmi == 0),
                                stop=(mmi == 31),
                            )
                            mmi += 1
                    ob = osbp.tile([128, 256], F32, tag="ob", name="ob")
                    nc.scalar.activation(ob[:, :], ps[:, :], ACTF.Copy)
                    if ct == 0:
                        nc.sync.dma_start(out=out[b, 0:31744], in_=ob[4:128, :])
                    elif ct == 15:
                        lo = 256 * (128 * 15 - 4)
                        nc.sync.dma_start(out=out[b, lo : lo + 32512], in_=ob[0:127, :])
                    else:
                        lo = 256 * (128 * ct - 4)
                        nc.sync.dma_start(out=out[b, lo : lo + 32768], in_=ob[:, :])
    nc.compile()
    return nc


_CACHE = {}


def _get_nc():
    if "nc" not in _CACHE:
        _CACHE["nc"] = build_nc()
    return _CACHE["nc"]


def kernel(mag, angle, invbasis, _trace=False, **_ignored):
    nc = _get_nc()
    mag = np.ascontiguousarray(np.asarray(mag, dtype=np.float32))
    angle = np.ascontiguousarray(np.asarray(angle, dtype=np.float32))
    invbasis = np.ascontiguousarray(np.asarray(invbasis, dtype=np.float32))
    in_maps = [
        {
            "mag": mag[B_LOCAL * i : B_LOCAL * (i + 1)],
            "angle": angle[B_LOCAL * i : B_LOCAL * (i + 1)],
            "invbasis": invbasis,
        }
        for i in range(N_CORES)
    ]
    res = run_bass_kernel_spmd(nc, in_maps, list(range(N_CORES)), trace=_trace)
    outs = [res.results[i]["out"] for i in range(N_CORES)]
    full = np.concatenate(outs, axis=0).reshape(16, 1, 523008)
    if _trace:
        return full, res
    return full
